# revision 1
# baseline (speedup 1.0000x reference)
"""Causal single-head attention (B=256, T=256, C=384, H=64) on 8 trn2 cores.

Data-parallel over batch: each core computes 32 batches independently.
Host prep: x is transposed to xt=[B, C, T] (so the contraction dim C lands on
SBUF partitions with no on-device transpose) and Wq is pre-scaled by 1/sqrt(H).
Per batch (fp32 storage, float32r matmuls = 4x fp32 PE rate, rel err ~3e-4):
  qT/kT/vT = W.T @ xT   (weights stationary)   [H=64, T=256]
  v   = transpose(vT) via PE                   [T, H]
  wei = (qT chunk).T @ kT                      [T, T]  (t0 row-tile only needs s0)
  causal mask: add -1e30 triangle to the two diagonal 128x128 blocks in PSUM
  p   = exp(wei) on ScalarE (no max-subtraction; logits are O(10)),
        accum_out gives row sums for free
  pT  = transpose(p) via PE                    [S, T]  ((s1,t0) block never used)
  out = (pT chunk).T @ v, scaled by 1/rowsum on the PSUM->SBUF copy
"""

import os
import sys

import numpy as np

for _p in ("/opt/trn_rl_repo",):
    if _p not in sys.path:
        sys.path.insert(0, _p)

B, T, C, H = 256, 256, 384, 64
N_CORES = 8
BPC = B // N_CORES  # batches per core
P = 128
NEG = -1e30

# matmul compute dtype: float32r streams at 4x the rate of float32 on the PE
# for free dims >= 256. Storage stays fp32; we bitcast at the matmul call.
MM_DT_NAME = os.environ.get("KERNEL_MM_DT", "float32r")

LAST_RESULT = None  # BassKernelResults of the most recent run (for test.py)


def _build_nc(bpc=BPC, repeats=1):
    import concourse.bacc as bacc
    import concourse.mybir as mybir
    import concourse.tile as tile
    from concourse.masks import make_causal_mask, make_identity

    f32 = mybir.dt.float32
    mm_dt = getattr(mybir.dt, MM_DT_NAME)


    nc = bacc.Bacc("TRN2", target_bir_lowering=False, debug=False)

    xt = nc.dram_tensor("xt", [bpc, C, T], mm_dt, kind="ExternalInput")
    wq = nc.dram_tensor("wq", [P, 3, H], mm_dt, kind="ExternalInput")
    wk = nc.dram_tensor("wk", [P, 3, H], mm_dt, kind="ExternalInput")
    wv = nc.dram_tensor("wv", [P, 3, H], mm_dt, kind="ExternalInput")
    out = nc.dram_tensor("out", [bpc, T, H], f32, kind="ExternalOutput")

    Exp = mybir.ActivationFunctionType.Exp
    add_op = mybir.AluOpType.add

    with tile.TileContext(nc) as tc:
        with (
            tc.tile_pool(name="consts", bufs=1) as consts,
            tc.tile_pool(name="sb", bufs=4) as sb,
            tc.tile_pool(name="ps_tr", bufs=2, space="PSUM") as ps_tr,
            tc.tile_pool(name="ps_q", bufs=1, space="PSUM") as ps_q,
            tc.tile_pool(name="ps_k", bufs=1, space="PSUM") as ps_k,
            tc.tile_pool(name="ps_v", bufs=1, space="PSUM") as ps_v,
            tc.tile_pool(name="ps_wei", bufs=2, space="PSUM") as ps_wei,
            tc.tile_pool(name="ps_out", bufs=1, space="PSUM") as ps_out,
        ):
            ident = consts.tile([P, P], f32)
            make_identity(nc, ident)
            tri = consts.tile([P, P], f32)
            make_causal_mask(nc, tri, mask_val=NEG)

            wq_sb = consts.tile([P, 3, H], mm_dt)
            nc.sync.dma_start(wq_sb, wq[:])
            wk_sb = consts.tile([P, 3, H], mm_dt)
            nc.sync.dma_start(wk_sb, wk[:])
            wv_sb = consts.tile([P, 3, H], mm_dt)
            nc.sync.dma_start(wv_sb, wv[:])

            # round-robin PSUM->SBUF copy engine to balance DVE/ACT load
            cp_state = [0]

            def copy(dst, src):
                cp_state[0] ^= 1
                if cp_state[0]:
                    nc.vector.tensor_copy(dst, src)
                else:
                    nc.scalar.copy(dst, src)

            import contextlib

            rep_ctx = (
                tc.For_i(0, repeats, 1, hint_engines=(mybir.EngineType.PE,
                                                      mybir.EngineType.DVE,
                                                      mybir.EngineType.Activation,
                                                      mybir.EngineType.SP))
                if repeats > 1
                else contextlib.nullcontext()
            )
            with rep_ctx:
              for b in range(bpc):
                  # ---- load xt[b] = x[b].T directly (host pre-transposed) ----
                  xT = sb.tile([P, 3, T], mm_dt, tag="xT")
                  xtb = xt[b].rearrange("(c p) t -> p c t", p=P)
                  for c in range(3):
                      nc.sync.dma_start(xT[:, c, :], xtb[:, c, :])

                  # ---- projections: qT/kT/vT = W.T @ xT  -> [64, 256] ----
                  q_ps = ps_q.tile([H, T], f32, tag="q")
                  k_ps = ps_k.tile([H, T], f32, tag="k")
                  v_ps = ps_v.tile([H, T], f32, tag="v")
                  for c in range(3):
                      nc.tensor.matmul(
                          q_ps, wq_sb[:, c, :], xT[:, c, :],
                          start=(c == 0), stop=(c == 2),
                      )
                  for c in range(3):
                      nc.tensor.matmul(
                          k_ps, wk_sb[:, c, :], xT[:, c, :],
                          start=(c == 0), stop=(c == 2),
                      )
                  for c in range(3):
                      nc.tensor.matmul(
                          v_ps, wv_sb[:, c, :], xT[:, c, :],
                          start=(c == 0), stop=(c == 2),
                      )
                  qT = sb.tile([H, T], mm_dt, tag="qT")
                  copy(qT, q_ps)
                  kT = sb.tile([H, T], mm_dt, tag="kT")
                  copy(kT, k_ps)
                  vT = sb.tile([H, T], f32, tag="vT")
                  copy(vT, v_ps)

                  # ---- v natural [t-part, h] via PE transpose of vT ----
                  v_sb = sb.tile([P, 2, H], mm_dt, tag="v")
                  for j in range(2):
                      tr = ps_tr.tile([P, P], f32, tag="tr")
                      nc.tensor.transpose(
                          tr[:, 0:H], vT[:, j * P : (j + 1) * P], ident[0:H, 0:H]
                      )
                      copy(v_sb[:, j, :], tr[:, 0:H])

                  # ---- wei = q @ k.T ; t0 tile only needs s in [0,128) ----
                  wei = ps_wei.tile([P, 2, T], f32, tag="wei")
                  nc.tensor.matmul(
                      wei[:, 0, 0:P], qT[:, 0:P], kT[:, 0:P],
                      start=True, stop=True,
                  )
                  nc.tensor.matmul(
                      wei[:, 1, :], qT[:, P : 2 * P], kT[:],
                      start=True, stop=True,
                  )
                  # causal mask: add NEG upper-triangle to the two diagonal blocks
                  nc.vector.tensor_tensor(
                      wei[:, 0, 0:P], wei[:, 0, 0:P], tri, add_op
                  )
                  nc.vector.tensor_tensor(
                      wei[:, 1, P:T], wei[:, 1, P:T], tri, add_op
                  )

                  # ---- softmax (no max subtraction; logits are O(10)) ----
                  p_sb = sb.tile([P, 2, T], f32, tag="p")
                  rowsum = sb.tile([P, 2], f32, tag="rowsum")
                  nc.scalar.activation(
                      p_sb[:, 0, 0:P], wei[:, 0, 0:P], Exp,
                      accum_out=rowsum[:, 0:1],
                  )
                  nc.scalar.activation(
                      p_sb[:, 1, :], wei[:, 1, :], Exp,
                      accum_out=rowsum[:, 1:2],
                  )
                  rinv = sb.tile([P, 2], f32, tag="rinv")
                  nc.vector.reciprocal(rinv, rowsum)

                  # ---- pT = transpose(p): chunks (i=t-tile, j=s-chunk) ----
                  pT = sb.tile([P, 2, T], mm_dt, tag="pT")
                  for (i, j) in ((0, 0), (1, 0), (1, 1)):
                      tr = ps_tr.tile([P, P], f32, tag="tr")
                      nc.tensor.transpose(tr, p_sb[:, i, j * P : (j + 1) * P], ident)
                      copy(pT[:, j, i * P : (i + 1) * P], tr)

                  # ---- out = p @ v   [t-part, h] ----
                  o_ps = ps_out.tile([P, 2, H], f32, tag="o")
                  nc.tensor.matmul(
                      o_ps[:, 0, :], pT[:, 0, 0:P], v_sb[:, 0, :],
                      start=True, stop=True,
                  )
                  nc.tensor.matmul(
                      o_ps[:, 1, :], pT[:, 0, P : 2 * P], v_sb[:, 0, :],
                      start=True, stop=False,
                  )
                  nc.tensor.matmul(
                      o_ps[:, 1, :], pT[:, 1, P : 2 * P], v_sb[:, 1, :],
                      start=False, stop=True,
                  )
                  o_sb = sb.tile([P, 2, H], f32, tag="o_sb")
                  nc.vector.tensor_tensor(
                      o_sb, o_ps,
                      rinv[:, :, None].to_broadcast((P, 2, H)),
                      mybir.AluOpType.mult,
                  )
                  nc.sync.dma_start(out[b].rearrange("(i p) h -> p i h", p=P), o_sb)

    nc.compile()
    return nc


def _prep_weight(w):
    # [C, H] -> [128, 3, H] so chunk c lives at [:, c, :]
    w = np.ascontiguousarray(np.asarray(w, dtype=np.float32))
    return np.ascontiguousarray(w.reshape(3, P, H).transpose(1, 0, 2))


def kernel(x, Wk, Wq, Wv):
    global LAST_RESULT
    from concourse.bass_utils import run_bass_kernel_spmd

    x = np.asarray(x, dtype=np.float32)
    xt = np.ascontiguousarray(x.transpose(0, 2, 1))
    scale = np.float32(H) ** np.float32(-0.5)
    wq_arr = _prep_weight(np.asarray(Wq, dtype=np.float32) * scale)
    wk_arr = _prep_weight(Wk)
    wv_arr = _prep_weight(Wv)

    nc = _build_nc()

    in_maps = [
        {
            "xt": np.ascontiguousarray(xt[c * BPC : (c + 1) * BPC]),
            "wq": wq_arr,
            "wk": wk_arr,
            "wv": wv_arr,
        }
        for c in range(N_CORES)
    ]
    trace = bool(int(os.environ.get("KERNEL_TRACE", "0")))
    res = run_bass_kernel_spmd(
        nc, in_maps, core_ids=list(range(N_CORES)), trace=trace
    )
    LAST_RESULT = res
    return np.concatenate([r["out"] for r in res.results], axis=0)



# revision 2
# speedup vs baseline: 6.0280x; 6.0280x over previous
"""Causal single-head attention (B=256, T=256, C=384, H=64) on 8 trn2 cores.

Data-parallel over batch: each core computes bpc=32 batches independently.

v2 design (vs baseline): fp16 matmul dtype (1 cycle/row at any output size,
halves DMA bytes), transposed-softmax formulation (weiT = k^T q with s on
partitions -> no PE transposes at all), natural-layout v projection
(x-stationary), row sums via an appended ones-column in the AV matmul,
causal masking as a multiplicative 0/1 triangle on the Pool engine (SBUF
fp16), and group-batched DMAs (G batches per DMA) to amortize per-DMA
overheads (~625ns HWDGE + ~565ns SEQ each).

Per batch:
  qT[h,t], kT[h,t] = Wq/Wk stationary @ xT moving     (2x3 matmuls, [64,2,256] PSUM)
  v[t,h]           = xT-block stationary @ Wv moving  (6 matmuls, [128,2,64] PSUM)
  weiT[s,t]        = kT-block stationary @ qT moving  (2 matmuls, [128,384] PSUM:
                     cols 0:256 = s0 x all t, cols 256:384 = s1 x t1)
  p = exp(weiT) on ACT (PSUM->SBUF fp16, one instruction; logits ~N(0,1) so no
      max-subtraction needed), diagonal blocks masked by 0/1 upper-triangle
      multiply on Pool.
  out[t, 0:64] + rowsum[t] = p-block stationary @ [v|1] moving (3 matmuls)
  out scaled by 1/rowsum on the PSUM->SBUF copy (DVE for t0, ACT for t1).
"""

import contextlib
import os
import sys

import numpy as np

for _p in ("/opt/trn_rl_repo",):
    if _p not in sys.path:
        sys.path.insert(0, _p)

B, T, C, H = 256, 256, 384, 64
N_CORES = 8
BPC = B // N_CORES  # batches per core
P = 128

LAST_RESULT = None  # BassKernelResults of the most recent run (for test.py)


def _build_nc(bpc=BPC, repeats=1, group=4):
    import concourse.bacc as bacc
    import concourse.mybir as mybir
    import concourse.tile as tile
    from concourse.masks import make_upper_triangular

    f32 = mybir.dt.float32
    f16 = mybir.dt.float16

    G = min(group, bpc)
    assert bpc % G == 0
    NG = bpc // G

    nc = bacc.Bacc("TRN2", target_bir_lowering=False, debug=False)

    xh = nc.dram_tensor("xh", [3, P, bpc, T], f16, kind="ExternalInput")
    wqk = nc.dram_tensor("wqk", [P, 3, 2, H], f16, kind="ExternalInput")
    wv = nc.dram_tensor("wv", [P, 3, H], f16, kind="ExternalInput")
    oh = nc.dram_tensor("oh", [P, bpc, 2, H], f16, kind="ExternalOutput")

    Exp = mybir.ActivationFunctionType.Exp
    Copy = mybir.ActivationFunctionType.Copy
    mult = mybir.AluOpType.mult

    xh_r = xh.rearrange("c p b t -> p c b t")

    with tile.TileContext(nc) as tc:
        with (
            tc.tile_pool(name="consts", bufs=1) as consts,
            tc.tile_pool(name="xg", bufs=2) as xg_pool,
            tc.tile_pool(name="og", bufs=2) as og_pool,
            tc.tile_pool(name="sb", bufs=8) as sb,
            tc.tile_pool(name="ps_qk", bufs=2, space="PSUM") as ps_qk,
            tc.tile_pool(name="ps_v", bufs=2, space="PSUM") as ps_v,
            tc.tile_pool(name="ps_wei", bufs=2, space="PSUM") as ps_wei,
            tc.tile_pool(name="ps_o", bufs=2, space="PSUM") as ps_o,
        ):
            wqk_sb = consts.tile([P, 3, 2, H], f16)
            nc.sync.dma_start(wqk_sb, wqk[:])
            wv_sb = consts.tile([P, 3, H], f16)
            nc.sync.dma_start(wv_sb, wv[:])
            tri01 = consts.tile([P, P], f16)
            make_upper_triangular(nc, tri01, val=1.0, diag=True)

            rep_ctx = (
                tc.For_i(0, repeats, 1, hint_engines=(mybir.EngineType.PE,
                                                      mybir.EngineType.DVE,
                                                      mybir.EngineType.Activation,
                                                      mybir.EngineType.Pool,
                                                      mybir.EngineType.SP))
                if repeats > 1
                else contextlib.nullcontext()
            )
            with rep_ctx:
              for g in range(NG):
                xg = xg_pool.tile([P, 3, G, T], f16, tag="xg")
                nc.sync.dma_start(xg, xh_r[:, :, g * G:(g + 1) * G, :])
                og = og_pool.tile([P, G, 2, H], f16, tag="og")
                for j in range(G):
                    # ---- projections ----
                    qk_ps = ps_qk.tile([H, 2, T], f32, tag="qk")
                    for w in range(2):
                        for c in range(3):
                            nc.tensor.matmul(
                                qk_ps[:, w, :], wqk_sb[:, c, w, :], xg[:, c, j, :],
                                start=(c == 0), stop=(c == 2),
                            )
                    v_ps = ps_v.tile([P, 2, H], f32, tag="v")
                    for i in range(2):
                        for c in range(3):
                            nc.tensor.matmul(
                                v_ps[:, i, :],
                                xg[:, c, j, i * P:(i + 1) * P], wv_sb[:, c, :],
                                start=(c == 0), stop=(c == 2),
                            )
                    qk_sb = sb.tile([H, 2, T], f16, tag="qk_sb")
                    nc.vector.tensor_copy(qk_sb, qk_ps)
                    v_aug = sb.tile([P, 2, H + 1], f16, tag="v_aug")
                    nc.gpsimd.memset(v_aug[:, :, H:H + 1], 1.0)
                    nc.scalar.copy(v_aug[:, :, 0:H], v_ps)

                    # ---- weiT = k^T q, [s, t] with s on partitions ----
                    wei_ps = ps_wei.tile([P, 3 * P], f32, tag="wei")
                    nc.tensor.matmul(
                        wei_ps[:, 0:T], qk_sb[:, 1, 0:P], qk_sb[:, 0, :],
                        start=True, stop=True,
                    )
                    nc.tensor.matmul(
                        wei_ps[:, T:3 * P], qk_sb[:, 1, P:T], qk_sb[:, 0, P:T],
                        start=True, stop=True,
                    )

                    # ---- softmax numerator (no max subtraction) ----
                    p_sb = sb.tile([P, 3 * P], f16, tag="p")
                    nc.scalar.activation(p_sb, wei_ps, Exp)
                    # causal mask: zero strict-lower triangle of the two
                    # diagonal (s,t) blocks (cols 0:128 and 256:384) in one
                    # strided op
                    p_diag = p_sb.rearrange("p (a q) -> p a q", q=P)[:, 0::2, :]
                    nc.gpsimd.tensor_tensor(
                        p_diag, p_diag,
                        tri01[:, None, :].to_broadcast((P, 2, P)), mult,
                    )

                    # ---- out = p @ [v|1] ----
                    o_ps = ps_o.tile([P, 2, H + 1], f32, tag="o")
                    nc.tensor.matmul(
                        o_ps[:, 0, :], p_sb[:, 0:P], v_aug[:, 0, :],
                        start=True, stop=True,
                    )
                    nc.tensor.matmul(
                        o_ps[:, 1, :], p_sb[:, P:T], v_aug[:, 0, :],
                        start=True, stop=False,
                    )
                    nc.tensor.matmul(
                        o_ps[:, 1, :], p_sb[:, T:3 * P], v_aug[:, 1, :],
                        start=False, stop=True,
                    )

                    # ---- normalize by rowsum (col H of o_ps) ----
                    rinv = sb.tile([P, 2], f32, tag="rinv")
                    nc.vector.reciprocal(rinv, o_ps[:, :, H])
                    nc.vector.tensor_tensor(
                        og[:, j, 0, :], o_ps[:, 0, 0:H],
                        rinv[:, 0:1].to_broadcast((P, H)), mult,
                    )
                    nc.scalar.activation(
                        og[:, j, 1, :], o_ps[:, 1, 0:H], Copy, scale=rinv[:, 1:2]
                    )
                nc.sync.dma_start(oh[:, g * G:(g + 1) * G, :, :], og)

    nc.compile()
    return nc


def _prep_inputs(x, Wk, Wq, Wv):
    """Full inputs -> per-core in_maps with the DRAM layouts above."""
    x = np.asarray(x, dtype=np.float32)
    scale = np.float32(H) ** np.float32(-0.5)
    wq = np.asarray(Wq, dtype=np.float32) * scale
    wk = np.asarray(Wk, dtype=np.float32)
    wv = np.asarray(Wv, dtype=np.float32)
    # wqk[p, c, w, h]
    wqk_arr = np.stack(
        [wq.reshape(3, P, H), wk.reshape(3, P, H)], axis=2
    ).transpose(1, 0, 2, 3)
    wqk_arr = np.ascontiguousarray(wqk_arr.astype(np.float16))
    wv_arr = np.ascontiguousarray(
        wv.reshape(3, P, H).transpose(1, 0, 2).astype(np.float16)
    )
    in_maps = []
    for cid in range(N_CORES):
        xc = x[cid * BPC:(cid + 1) * BPC]  # [bpc, T, C]
        xh = xc.reshape(BPC, T, 3, P).transpose(2, 3, 0, 1)  # [3, P, bpc, T]
        in_maps.append({
            "xh": np.ascontiguousarray(xh.astype(np.float16)),
            "wqk": wqk_arr,
            "wv": wv_arr,
        })
    return in_maps


def _assemble_output(results):
    """Per-core oh [P, bpc, 2, H] fp16 -> full out [B, T, H] fp32."""
    outs = []
    for r in results:
        oh = np.asarray(r["oh"], dtype=np.float32)  # [P, bpc, 2, H]
        outs.append(oh.transpose(1, 2, 0, 3).reshape(BPC, T, H))
    return np.concatenate(outs, axis=0)


def kernel(x, Wk, Wq, Wv):
    global LAST_RESULT
    from concourse.bass_utils import run_bass_kernel_spmd

    in_maps = _prep_inputs(x, Wk, Wq, Wv)
    nc = _build_nc()
    trace = bool(int(os.environ.get("KERNEL_TRACE", "0")))
    res = run_bass_kernel_spmd(
        nc, in_maps, core_ids=list(range(N_CORES)), trace=trace
    )
    LAST_RESULT = res
    return _assemble_output(res.results)
